# revision 1
# baseline (speedup 1.0000x reference)
"""Trainium2 Bass kernel for nn_MultiHeadSelfAttention_36472862277880.

Sparse attention (local window 128, global stride 64, causal) with RoPE.
Sharding: 8 cores = 4 batches x 2 head-groups (8 heads each).
Per core: QKV projection (fp32r matmuls), k-major sparse attention
(window + global-key + global-query parts, single softmax via ones-column),
out-projection partial; host sums the two head-group partials.

Self-contained: hardcodes all shapes; only imports the system concourse repo.
"""
import sys

if "/opt/trn_rl_repo" not in sys.path:
    sys.path.insert(0, "/opt/trn_rl_repo")

import numpy as np
import ml_dtypes

import concourse.bass as bass
import concourse.bacc as bacc
import concourse.tile as tile
from concourse import mybir
from concourse.bass_utils import run_bass_kernel_spmd
from concourse.masks import make_identity

F32 = mybir.dt.float32
F32R = mybir.dt.float32r
BF16 = mybir.dt.bfloat16

B, T, D, H, DH = 4, 2048, 1024, 16, 64
HALF = T // 2
NCORES = 8
EXP_SCALE = 0.125

_cache = {}


def _build():
    nc = bacc.Bacc("TRN2", target_bir_lowering=False, debug=False, num_devices=1)

    x_d = nc.dram_tensor("x", [T, D], F32, kind="ExternalInput").ap()
    wqk_d = nc.dram_tensor("wqk", [128, 8, 8, 128], F32, kind="ExternalInput").ap()
    wv_d = nc.dram_tensor("wv", [128, 8, 512], F32, kind="ExternalInput").ap()
    wout_d = nc.dram_tensor("wout", [128, 4, 1024], F32, kind="ExternalInput").ap()
    cos4_d = nc.dram_tensor("cos4", [128, T], F32, kind="ExternalInput").ap()
    sin4_d = nc.dram_tensor("sin4", [128, T], F32, kind="ExternalInput").ap()
    cos4g_d = nc.dram_tensor("cos4g", [128, 32], F32, kind="ExternalInput").ap()
    sin4g_d = nc.dram_tensor("sin4g", [128, 32], F32, kind="ExternalInput").ap()
    mwin_d = nc.dram_tensor("mwin", [128, 512], BF16, kind="ExternalInput").ap()
    mglob_d = nc.dram_tensor("mglob", [32, T], BF16, kind="ExternalInput").ap()
    mp2_d = nc.dram_tensor("mp2", [128, 512], BF16, kind="ExternalInput").ap()
    out_d = nc.dram_tensor("out", [T, D], F32, kind="ExternalOutput").ap()

    EXP = mybir.ActivationFunctionType.Exp

    with tile.TileContext(nc) as tc:
        from contextlib import ExitStack
        with ExitStack() as ctx:
            sb = ctx.enter_context(tc.tile_pool(name="sb", bufs=1))
            ps = ctx.enter_context(tc.tile_pool(name="ps", bufs=1, space="PSUM"))

            # ---------- constants ----------
            ident = sb.tile([128, 128], F32, tag="ident", name="ident")
            make_identity(nc, ident[:])
            onesf = sb.tile([1, 64], F32, tag="onesf", name="onesf")
            nc.vector.memset(onesf[:], 1.0)
            ones64 = sb.tile([1, 64], F32R, tag="ones64", name="ones64")
            nc.vector.tensor_copy(ones64[:], onesf[:])
            mwin = sb.tile([128, 512], BF16, tag="mwin", name="mwin")
            nc.sync.dma_start(out=mwin[:], in_=mwin_d)
            mglob = sb.tile([32, T], BF16, tag="mglob", name="mglob")
            nc.sync.dma_start(out=mglob[:], in_=mglob_d)
            mp2 = sb.tile([128, 512], BF16, tag="mp2", name="mp2")
            nc.sync.dma_start(out=mp2[:], in_=mp2_d)
            cos4g = sb.tile([128, 32], F32, tag="cos4g", name="cos4g")
            nc.sync.dma_start(out=cos4g[:], in_=cos4g_d)
            sin4g = sb.tile([128, 32], F32, tag="sin4g", name="sin4g")
            nc.sync.dma_start(out=sin4g[:], in_=sin4g_d)

            wvr = sb.tile([128, 8, 512], F32R, tag="wvr", name="wvr")
            for c2 in range(4):
                wvs = sb.tile([128, 2, 512], F32, tag="wvs", name=f"wvs{c2}")
                nc.sync.dma_start(out=wvs[:], in_=wv_d[:, 2 * c2:2 * c2 + 2, :])
                nc.vector.tensor_copy(wvr[:, 2 * c2:2 * c2 + 2, :], wvs[:])

            wor = []
            for cc in range(4):
                wos = sb.tile([128, 1024], F32, tag="wos", bufs=1, name=f"wos{cc}")
                nc.sync.dma_start(out=wos[:], in_=wout_d[:, cc, :])
                w = sb.tile([128, 1024], F32R, tag="wor", bufs=4, name=f"wor{cc}")
                nc.vector.tensor_copy(w[:], wos[:])
                wor.append(w)

            def load_wqk(h, tag_sfx=""):
                wqs = sb.tile([128, 8, 128], F32, tag="wqs", bufs=1,
                              name=f"wqs{tag_sfx}{h}")
                nc.sync.dma_start(out=wqs[:], in_=wqk_d[:, h, :, :])
                wqr = sb.tile([128, 8, 128], F32R, tag="wqr", bufs=2,
                              name=f"wqr{tag_sfx}{h}")
                nc.scalar.copy(wqr[:], wqs[:])
                return wqr

            def rope_psum(pq, qkr, cos_t, sin_t, off, width, sfx):
                """pq psum (128,w) rows [Qe,Ke,Qo,Ko] -> qkr[:, off:off+w] f32r.
                ACT evicts psum -> sbuf so the DVE muls run in 2x mode."""
                sl = slice(off, off + width)
                rw = sb.tile([128, 512], F32, tag="qraw", bufs=2,
                             name=f"qraw{sfx}")
                nc.scalar.copy(rw[:, 0:width], pq[:, 0:width])
                E = rw[0:64, 0:width]
                O = rw[64:128, 0:width]
                re_f = qkr[0:64, sl].bitcast(F32)
                ro_f = qkr[64:128, sl].bitcast(F32)
                tt = sb.tile([128, 512], F32, tag="rt2", bufs=2,
                             name=f"rt{sfx}")
                t2 = tt[0:64, 0:width]
                t4 = tt[64:128, 0:width]
                nc.vector.tensor_mul(re_f, E, cos_t[0:64, sl])
                nc.vector.tensor_mul(t2, O, sin_t[64:128, sl])
                nc.vector.tensor_sub(qkr[0:64, sl], re_f, t2)
                nc.vector.tensor_mul(ro_f, E, sin_t[0:64, sl])
                nc.vector.tensor_mul(t4, O, cos_t[64:128, sl])
                nc.vector.tensor_add(qkr[64:128, sl], ro_f, t4)

            # ---------- early pass: global tokens (t = 64m) ----------
            xgs = sb.tile([32, 1024], F32, tag="xstage", bufs=4, name="xgs")
            nc.sync.dma_start(out=xgs[:], in_=x_d[0::64, :])
            pxg = ps.tile([128, 8, 32], F32, tag="psA", bufs=2, name="pxg")
            for dc in range(8):
                nc.tensor.transpose(pxg[:, dc, :], xgs[:, 128 * dc:128 * dc + 128],
                                    ident[0:32, 0:32])
            xgt = sb.tile([128, 8, 32], F32R, tag="xgt", name="xgt")
            nc.vector.tensor_copy(xgt[:], pxg[:])

            pvg = ps.tile([128, 512], F32, tag="psB", bufs=2, name="pvg")
            for dc in range(8):
                nc.tensor.matmul(pvg[0:32, :], xgt[:, dc, :], wvr[:, dc, :],
                                 start=(dc == 0), stop=(dc == 7))
            vg = sb.tile([32, 8, 65], BF16, tag="vg", name="vg")
            nc.scalar.copy(vg[:, :, 0:64],
                           pvg[0:32, :].rearrange("p (a b) -> p a b", a=8))
            nc.vector.memset(vg[:, :, 64], 1.0)

            qg, kg = [], []
            for h in range(8):
                wqr = load_wqk(h, "e")
                pqg = ps.tile([128, 512], F32, tag="psQK", bufs=2, name=f"pqg{h}")
                for dc in range(8):
                    nc.tensor.matmul(pqg[:, 0:32], wqr[:, dc, :], xgt[:, dc, :],
                                     start=(dc == 0), stop=(dc == 7))
                qkrg = sb.tile([128, 1024], F32R, tag="qkr", bufs=2,
                               name=f"qkrg{h}")
                rope_psum(pqg[:, 0:32], qkrg, cos4g[:], sin4g[:], 0, 32,
                          f"g{h}")
                qgh = sb.tile([64, 32], F32R, tag="qg", bufs=8, name=f"qg{h}")
                kgh = sb.tile([64, 32], F32R, tag="kg", bufs=8, name=f"kg{h}")
                nc.vector.tensor_copy(qgh[0:32, :], qkrg[0:32, 0:32])
                nc.vector.tensor_copy(qgh[32:64, :], qkrg[64:96, 0:32])
                nc.vector.tensor_copy(kgh[0:32, :], qkrg[32:64, 0:32])
                nc.vector.tensor_copy(kgh[32:64, :], qkrg[96:128, 0:32])
                qg.append(qgh)
                kg.append(kgh)

            p2sb = [sb.tile([32, 65], F32, tag="p2sb", bufs=8, name=f"p2sb{h}")
                    for h in range(8)]
            ktb = [sb.tile([64, 128], F32R, tag="ktb", bufs=8, name=f"ktb{h}")
                   for h in range(8)]
            eb_prev = [None] * 8
            vaug_prev7 = None

            # ---------- half loop ----------
            for Hh in range(2):
                t0 = HALF * Hh

                xt = [sb.tile([128, 1024], F32R, tag="xt", bufs=8,
                              name=f"xt{Hh}_{dc}") for dc in range(8)]
                for tc in range(2):
                    stages = []
                    for subq in range(4):
                        xs = sb.tile([128, 1024], F32, tag="xstage", bufs=4,
                                     name=f"xs{Hh}_{tc}_{subq}")
                        r0 = t0 + 512 * tc + 128 * subq
                        nc.sync.dma_start(out=xs[:], in_=x_d[r0:r0 + 128, :])
                        stages.append(xs)
                    for dc in range(8):
                        ptr = ps.tile([128, 512], F32, tag="psA", bufs=2,
                                      name=f"ptr{Hh}_{tc}_{dc}")
                        for subq in range(4):
                            nc.tensor.transpose(
                                ptr[:, 128 * subq:128 * subq + 128],
                                stages[subq][:, 128 * dc:128 * dc + 128],
                                ident[:, :])
                        nc.scalar.copy(xt[dc][:, 512 * tc:512 * tc + 512],
                                       ptr[:])

                vaug = []
                for jl in range(8):
                    pv = ps.tile([128, 512], F32, tag="psB", bufs=2,
                                 name=f"pv{Hh}_{jl}")
                    for dc in range(8):
                        nc.tensor.matmul(pv[:], xt[dc][:, 128 * jl:128 * jl + 128],
                                         wvr[:, dc, :],
                                         start=(dc == 0), stop=(dc == 7))
                    va = sb.tile([128, 8, 65], BF16, tag="vaug", bufs=9,
                                 name=f"va{Hh}_{jl}")
                    nc.scalar.copy(va[:, :, 0:64],
                                   pv[:].rearrange("p (a b) -> p a b", a=8))
                    nc.vector.memset(va[:, :, 64], 1.0)
                    vaug.append(va)

                cosh = sb.tile([128, 1024], F32, tag="cosh", name=f"cosh{Hh}")
                nc.sync.dma_start(out=cosh[:], in_=cos4_d[:, t0:t0 + HALF])
                sinh = sb.tile([128, 1024], F32, tag="sinh", name=f"sinh{Hh}")
                nc.sync.dma_start(out=sinh[:], in_=sin4_d[:, t0:t0 + HALF])

                attn = [sb.tile([128, 1024], F32R, tag="attn", bufs=4,
                                name=f"attn{Hh}_{cc}") for cc in range(4)]

                for h in range(8):
                    wqr = load_wqk(h, f"m{Hh}")
                    qkr = sb.tile([128, 1024], F32R, tag="qkr", bufs=2,
                                  name=f"qkr{Hh}_{h}")
                    for tc in range(2):
                        pqk = ps.tile([128, 512], F32, tag="psQK", bufs=2,
                                      name=f"pqk{Hh}_{h}_{tc}")
                        for dc in range(8):
                            nc.tensor.matmul(pqk[:], wqr[:, dc, :],
                                             xt[dc][:, 512 * tc:512 * tc + 512],
                                             start=(dc == 0), stop=(dc == 7))
                        rope_psum(pqk[:], qkr, cosh[:], sinh[:], 512 * tc, 512,
                                  f"m{Hh}_{h}_{tc}")
                    qt = sb.tile([64, 1024], F32R, tag="qt", bufs=2,
                                 name=f"qt{Hh}_{h}")
                    kt = sb.tile([64, 1024], F32R, tag="kt", bufs=2,
                                 name=f"kt{Hh}_{h}")
                    nc.gpsimd.tensor_copy(qt[0:32, :], qkr[0:32, :])
                    nc.gpsimd.tensor_copy(qt[32:64, :], qkr[64:96, :])
                    nc.gpsimd.tensor_copy(kt[0:32, :], qkr[32:64, :])
                    nc.gpsimd.tensor_copy(kt[32:64, :], qkr[96:128, :])
                    if Hh == 0:
                        nc.gpsimd.tensor_copy(ktb[h][:], kt[:, 896:1024])

                    # window scores
                    ej = []
                    for jp in range(4):
                        j0, j1 = 2 * jp, 2 * jp + 1
                        w1 = 256 if j1 < 7 else 128
                        pw = ps.tile([128, 512], F32, tag="psA", bufs=2,
                                     name=f"pw{Hh}_{h}_{jp}")
                        nc.tensor.matmul(pw[:, 0:256],
                                         kt[:, 128 * j0:128 * j0 + 128],
                                         qt[:, 128 * j0:128 * j0 + 256],
                                         start=True, stop=False)
                        nc.tensor.matmul(pw[:, 256:256 + w1],
                                         kt[:, 128 * j1:128 * j1 + 128],
                                         qt[:, 128 * j1:128 * j1 + w1],
                                         start=False, stop=True)
                        et = sb.tile([128, 512], BF16, tag="exptmp", bufs=3,
                                     name=f"et{Hh}_{h}_{jp}")
                        nc.scalar.activation(et[:, 0:256 + w1], pw[:, 0:256 + w1],
                                             EXP, scale=EXP_SCALE)
                        ep = sb.tile([128, 512], BF16, tag="ej", bufs=5,
                                     name=f"e{Hh}_{h}_{jp}")
                        nc.vector.tensor_mul(ep[:, 0:256 + w1],
                                             et[:, 0:256 + w1],
                                             mwin[:, 0:256 + w1])
                        ej.append(ep[:, 0:256])
                        ej.append(ep[:, 256:512])

                    if Hh == 1:
                        pb = ps.tile([128, 512], F32, tag="psA", bufs=2,
                                     name=f"pb{h}")
                        nc.tensor.matmul(pb[:, 0:128], ktb[h][:], qt[:, 0:128],
                                         start=True, stop=True)
                        etb = sb.tile([128, 512], BF16, tag="exptmp", bufs=3,
                                      name=f"etb{h}")
                        nc.scalar.activation(etb[:, 0:128], pb[:, 0:128],
                                             EXP, scale=EXP_SCALE)
                        eb = sb.tile([128, 128], BF16, tag="eb", bufs=2,
                                     name=f"eb{h}")
                        nc.vector.tensor_mul(eb[:], etb[:, 0:128],
                                             mwin[:, 128:256])
                        eb_prev[h] = eb

                    # global-key scores
                    eglob = sb.tile([32, 1024], BF16, tag="eglob", name=f"eg{Hh}_{h}")
                    for tc in range(2):
                        pg = ps.tile([128, 512], F32, tag="psA", bufs=2,
                                     name=f"pg{Hh}_{h}_{tc}")
                        nc.tensor.matmul(pg[0:32, :], kg[h][:],
                                         qt[:, 512 * tc:512 * tc + 512],
                                         start=True, stop=True)
                        etg = sb.tile([128, 512], BF16, tag="exptmp", bufs=3,
                                      name=f"etg{Hh}_{h}_{tc}")
                        nc.scalar.activation(etg[0:32, :], pg[0:32, :],
                                             EXP, scale=EXP_SCALE)
                        nc.vector.tensor_mul(
                            eglob[:, 512 * tc:512 * tc + 512], etg[0:32, :],
                            mglob[:, t0 + 512 * tc:t0 + 512 * tc + 512])

                    # global-query (p2) scores
                    pp = ps.tile([128, 512], F32, tag="psA", bufs=2,
                                 name=f"pp{Hh}_{h}")
                    for jl in range(8):
                        nc.tensor.matmul(pp[:, 32 * jl:32 * jl + 32],
                                         kt[:, 128 * jl:128 * jl + 128], qg[h][:],
                                         start=(jl == 0), stop=(jl == 7))
                    etp = sb.tile([128, 512], BF16, tag="exptmp", bufs=3,
                                  name=f"etp{Hh}_{h}")
                    nc.scalar.activation(etp[:, 0:256], pp[:, 0:256],
                                         EXP, scale=EXP_SCALE)
                    ep2 = sb.tile([128, 256], BF16, tag="ep2", bufs=2,
                                  name=f"ep2{Hh}_{h}")
                    nc.vector.tensor_mul(ep2[:], etp[:, 0:256],
                                         mp2[:, 256 * Hh:256 * Hh + 256])

                    pc = ps.tile([32, 65], F32, tag="psC", bufs=1,
                                 name=f"pc{Hh}_{h}")
                    for jl in range(8):
                        nc.tensor.matmul(pc[:], ep2[:, 32 * jl:32 * jl + 32],
                                         vaug[jl][:, h, :],
                                         start=(jl == 0), stop=(jl == 7))
                    if Hh == 0:
                        nc.vector.tensor_copy(p2sb[h][:], pc[:])
                    else:
                        nc.vector.tensor_add(p2sb[h][:], p2sb[h][:], pc[:])

                    # AV u-chunks
                    cc, hh = h // 2, h % 2
                    for c in range(2):
                        U = ps.tile([128, 512], F32, tag="psB", bufs=2,
                                    name=f"U{Hh}_{h}_{c}")
                        if c == 0:
                            parts = [(0, 0, 256, 0), (1, 128, 384, 0),
                                     (2, 256, 512, 0), (3, 384, 512, 0)]
                        else:
                            parts = [(4, 0, 256, 0), (3, 0, 128, 128),
                                     (5, 128, 384, 0), (6, 256, 512, 0),
                                     (7, 384, 512, 0)]
                        first = True
                        for (jl, u0, u1, e0c) in parts:
                            wdt = u1 - u0
                            nc.tensor.matmul(U[0:65, u0:u1], vaug[jl][:, h, :],
                                             ej[jl][:, e0c:e0c + wdt],
                                             start=first, stop=False)
                            first = False
                        if Hh == 1 and c == 0:
                            nc.tensor.matmul(U[0:65, 0:128],
                                             vaug_prev7[:, h, :],
                                             eb_prev[h][:],
                                             start=False, stop=False)
                        nc.tensor.matmul(U[0:65, :], vg[:, h, :],
                                         eglob[:, 512 * c:512 * c + 512],
                                         start=False, stop=True)
                        zr = sb.tile([1, 512], F32R, tag="zr", bufs=2,
                                     name=f"zr{Hh}_{h}_{c}")
                        with nc.allow_low_precision(reason="softmax recip"):
                            nc.vector.reciprocal(zr[:], U[64:65, :])
                        pz = ps.tile([128, 512], F32, tag="psT", bufs=1,
                                     name=f"pz{Hh}_{h}_{c}")
                        nc.tensor.matmul(pz[0:64, :], ones64[:], zr[:],
                                         start=True, stop=True)
                        bc = sb.tile([64, 512], F32, tag="bc", bufs=1,
                                     name=f"bc{Hh}_{h}_{c}")
                        nc.scalar.copy(bc[:], pz[0:64, :])
                        nc.vector.tensor_mul(
                            attn[cc][64 * hh:64 * hh + 64,
                                     512 * c:512 * c + 512],
                            U[0:64, :], bc[:])

                    # p2 finalize for this half's global queries
                    m0 = 16 * Hh
                    rz2 = sb.tile([32, 1], F32, tag="rz2", bufs=2,
                                  name=f"rz2{Hh}_{h}")
                    nc.vector.reciprocal(rz2[:], p2sb[h][:, 64:65])
                    p2n = sb.tile([32, 64], F32, tag="p2n", bufs=2,
                                  name=f"p2n{Hh}_{h}")
                    nc.vector.tensor_scalar_mul(p2n[:], p2sb[h][:, 0:64], rz2[:])
                    pt = ps.tile([128, 512], F32, tag="psT", bufs=1,
                                 name=f"pt{Hh}_{h}")
                    nc.tensor.transpose(pt[0:64, 0:32], p2n[:],
                                        ident[0:32, 0:32])
                    nc.vector.tensor_copy(
                        attn[cc][64 * hh:64 * hh + 64, 0::64],
                        pt[0:64, m0:m0 + 16])

                vaug_prev7 = vaug[7]

                # out projection for this half
                for tc in range(8):
                    for nck in range(2):
                        po = ps.tile([128, 512], F32, tag="psB", bufs=2,
                                     name=f"po{Hh}_{tc}_{nck}")
                        for cci in range(4):
                            nc.tensor.matmul(
                                po[:], attn[cci][:, 128 * tc:128 * tc + 128],
                                wor[cci][:, 512 * nck:512 * nck + 512],
                                start=(cci == 0), stop=(cci == 3))
                        os_ = sb.tile([128, 512], F32, tag="ostage", bufs=3,
                                      name=f"os{Hh}_{tc}_{nck}")
                        nc.scalar.copy(os_[:], po[:])
                        nc.sync.dma_start(
                            out=out_d[t0 + 128 * tc:t0 + 128 * tc + 128,
                                      512 * nck:512 * nck + 512],
                            in_=os_[:])

    nc.compile()
    return nc


def _prep_inputs(x, W_qkv, W_out):
    bf = ml_dtypes.bfloat16
    pos = np.arange(T, dtype=np.float32)[:, None]
    half = DH // 2
    inv_freq = 1.0 / (10000.0 ** (np.arange(half, dtype=np.float32) / half))
    ang = pos * inv_freq[None, :]
    cosT = np.cos(ang).T.astype(np.float32)
    sinT = np.sin(ang).T.astype(np.float32)
    cos4 = np.ascontiguousarray(np.tile(cosT, (4, 1)))
    sin4 = np.ascontiguousarray(np.tile(sinT, (4, 1)))
    cos4g = np.ascontiguousarray(cos4[:, ::64])
    sin4g = np.ascontiguousarray(sin4[:, ::64])

    s = np.arange(128)[:, None]
    u = np.arange(256)[None, :]
    mwin1 = ((u >= s) & ((u <= s + 127) | (s % 64 == 0))).astype(bf)
    mwin = np.concatenate([mwin1, mwin1], axis=1)
    m = np.arange(32)[:, None]
    q = np.arange(T)[None, :]
    mglob = (q >= 128 * (m // 2 + 2)).astype(bf)
    sj = np.arange(128)[:, None, None]
    jj = np.arange(16)[None, :, None]
    mm_ = np.arange(32)[None, None, :]
    mp2 = (64 * mm_ >= 128 * jj + sj).astype(bf).reshape(128, 512)

    in_maps = []
    for core in range(NCORES):
        b, g = core // 2, core % 2
        whs = []
        for h in range(8):
            hg = 8 * g + h
            wq = W_qkv[:, 64 * hg:64 * hg + 64]
            wk = W_qkv[:, D + 64 * hg:D + 64 * hg + 64]
            wh = np.concatenate([wq[:, 0::2], wk[:, 0::2],
                                 wq[:, 1::2], wk[:, 1::2]], axis=1)
            whs.append(wh.reshape(8, 128, 128))
        wqk = np.ascontiguousarray(
            np.stack(whs, 0).transpose(2, 0, 1, 3)).astype(np.float32)
        wv = np.ascontiguousarray(
            W_qkv[:, 2 * D + 512 * g:2 * D + 512 * (g + 1)]
            .reshape(8, 128, 512).transpose(1, 0, 2)).astype(np.float32)
        wout = np.ascontiguousarray(
            W_out[512 * g:512 * (g + 1)]
            .reshape(4, 128, 1024).transpose(1, 0, 2)).astype(np.float32)
        in_maps.append({
            "x": np.ascontiguousarray(x[b]).astype(np.float32),
            "wqk": wqk, "wv": wv, "wout": wout,
            "cos4": cos4, "sin4": sin4, "cos4g": cos4g, "sin4g": sin4g,
            "mwin": mwin, "mglob": mglob, "mp2": mp2,
        })
    return in_maps


def kernel(x, W_qkv, W_out, b_out):
    x = np.asarray(x, dtype=np.float32)
    W_qkv = np.asarray(W_qkv, dtype=np.float32)
    W_out = np.asarray(W_out, dtype=np.float32)
    b_out = np.asarray(b_out, dtype=np.float32)

    if "nc" not in _cache:
        _cache["nc"] = _build()
    nc = _cache["nc"]

    in_maps = _prep_inputs(x, W_qkv, W_out)
    res = run_bass_kernel_spmd(nc, in_maps, core_ids=list(range(NCORES)))

    out = np.zeros((B, T, D), dtype=np.float32)
    for core in range(NCORES):
        out[core // 2] += res.results[core]["out"]
    out += b_out[None, None, :]
    return out



# revision 8
# speedup vs baseline: 1.9714x; 1.9714x over previous
"""Trainium2 Bass kernel for nn_MultiHeadSelfAttention_36472862277880 (v2).

Sparse attention (local window 128, global stride 64, causal) with RoPE.
Sharding: 8 cores = 4 batches x 2 head-groups (8 heads each).

v2 design (vs v1 baseline):
- all-bf16 matmul datapath (x, W, scores, AV, out-proj)
- host pre-transposes x (kills on-chip PE transposes + evictions)
- head-pair packed projection psums -> full-128-partition RoPE (6 DVE ops
  per pair instead of 6 per head at half width)
- per-head-contiguous QT/KT tiles via 8 int32-bitcast extraction copies
- batched exps over (128,1024)/(64,1024) psum tiles
- reciprocal_approx_fast + gpsimd partition_broadcast for softmax norm
- gpsimd offloads (masks, broadcast, memsets) to keep DVE below TensorE

Self-contained: hardcodes all shapes; only imports the system concourse repo.
"""
import sys

if "/opt/trn_rl_repo" not in sys.path:
    sys.path.insert(0, "/opt/trn_rl_repo")

import numpy as np
import ml_dtypes

import concourse.bass as bass
import concourse.bacc as bacc
import concourse.tile as tile
from concourse import mybir
from concourse.bass_utils import run_bass_kernel_spmd
from concourse.masks import make_identity

F32 = mybir.dt.float32
BF16 = mybir.dt.bfloat16
I32 = mybir.dt.int32

B, T, D, H, DH = 4, 2048, 1024, 16, 64
HALF = T // 2
NCORES = 8
EXP_SCALE = 0.125
EXP = mybir.ActivationFunctionType.Exp

_cache = {}


def _build():
    nc = bacc.Bacc("TRN2", target_bir_lowering=False, debug=False, num_devices=1)

    xt_d = nc.dram_tensor("xt", [128, 8, T], BF16, kind="ExternalInput").ap()
    xg_d = nc.dram_tensor("xg", [128, 8, 32], BF16, kind="ExternalInput").ap()
    wqk_d = nc.dram_tensor("wqk", [128, 8, 4, 2, 128], BF16,
                           kind="ExternalInput").ap()
    wv_d = nc.dram_tensor("wv", [128, 8, 512], BF16, kind="ExternalInput").ap()
    wout_d = nc.dram_tensor("wout", [128, 4, 1024], BF16,
                            kind="ExternalInput").ap()
    cos_d = nc.dram_tensor("cosd", [128, T], BF16, kind="ExternalInput").ap()
    sin_d = nc.dram_tensor("sind", [128, T], BF16, kind="ExternalInput").ap()
    cosg_d = nc.dram_tensor("cosg", [128, 32], BF16, kind="ExternalInput").ap()
    sing_d = nc.dram_tensor("sing", [128, 32], BF16, kind="ExternalInput").ap()
    mwin_d = nc.dram_tensor("mwin", [128, 1024], BF16, kind="ExternalInput").ap()
    mglob_d = nc.dram_tensor("mglob", [64, T], BF16, kind="ExternalInput").ap()
    mp2t_d = nc.dram_tensor("mp2t", [64, T], BF16, kind="ExternalInput").ap()
    idb_d = nc.dram_tensor("idb", [128, 32], BF16, kind="ExternalInput").ap()
    out_d = nc.dram_tensor("out", [T, D], BF16, kind="ExternalOutput").ap()

    with tile.TileContext(nc) as tc:
        from contextlib import ExitStack
        with ExitStack() as ctx:
            sb = ctx.enter_context(tc.tile_pool(name="sb", bufs=1))
            ps = ctx.enter_context(tc.tile_pool(name="ps", bufs=1, space="PSUM"))

            # ---------- resident constants ----------
            ident = sb.tile([128, 128], F32, tag="ident", name="ident")
            make_identity(nc, ident[:])
            idb = sb.tile([128, 32], BF16, tag="idb", name="idb")
            nc.sync.dma_start(out=idb[:], in_=idb_d)
            mwin = sb.tile([128, 1024], BF16, tag="mwin", name="mwin")
            nc.sync.dma_start(out=mwin[:], in_=mwin_d)
            mglob = sb.tile([64, T], BF16, tag="mglob", name="mglob")
            nc.sync.dma_start(out=mglob[:], in_=mglob_d)
            mp2t = sb.tile([64, T], BF16, tag="mp2t", name="mp2t")
            nc.sync.dma_start(out=mp2t[:], in_=mp2t_d)
            cosT = sb.tile([128, T], BF16, tag="cosT", name="cosT")
            nc.sync.dma_start(out=cosT[:], in_=cos_d)
            sinT = sb.tile([128, T], BF16, tag="sinT", name="sinT")
            nc.sync.dma_start(out=sinT[:], in_=sin_d)
            cosg = sb.tile([128, 32], BF16, tag="cosg", name="cosg")
            nc.sync.dma_start(out=cosg[:], in_=cosg_d)
            sing = sb.tile([128, 32], BF16, tag="sing", name="sing")
            nc.sync.dma_start(out=sing[:], in_=sing_d)

            wqk = sb.tile([128, 8, 4, 2, 128], BF16, tag="wqk", name="wqk")
            nc.sync.dma_start(out=wqk[:], in_=wqk_d)
            wv = sb.tile([128, 8, 512], BF16, tag="wv", name="wv")
            nc.sync.dma_start(out=wv[:], in_=wv_d)
            wout = sb.tile([128, 4, 1024], BF16, tag="wout", name="wout")
            nc.sync.dma_start(out=wout[:], in_=wout_d)

            xg = sb.tile([128, 8, 32], BF16, tag="xg", name="xg")
            nc.sync.dma_start(out=xg[:], in_=xg_d)
            xt = sb.tile([128, 8, T], BF16, tag="xt", name="xt")
            for q4 in range(4):
                nc.sync.dma_start(out=xt[:, :, 512 * q4:512 * q4 + 512],
                                  in_=xt_d[:, :, 512 * q4:512 * q4 + 512])

            def extract(QT, KT, RE, RO, w):
                """RE=[reQ_a;reK_a;reQ_b;reK_b], RO likewise ->
                QT=[reQ_a;roQ_a;reQ_b;roQ_b], KT same for K. w = col count."""
                wi = w // 2  # int32 cols
                pairs = [(QT, 0, RE, 0), (QT, 32, RO, 0),
                         (QT, 64, RE, 64), (QT, 96, RO, 64),
                         (KT, 0, RE, 32), (KT, 32, RO, 32),
                         (KT, 64, RE, 96), (KT, 96, RO, 96)]
                for n, (dst, dr, src, sr) in enumerate(pairs):
                    eng = nc.scalar if n in (3, 7) else nc.vector
                    if eng is nc.scalar:
                        nc.scalar.copy(dst[dr:dr + 32, 0:w], src[sr:sr + 32, 0:w])
                    else:
                        nc.vector.tensor_copy(
                            dst[dr:dr + 32, 0:w].bitcast(I32),
                            src[sr:sr + 32, 0:w].bitcast(I32))

            # ---------- early pass: global tokens (t = 64m) ----------
            # V of global tokens, packed per head at partition 32*(h%2)
            pvg = ps.tile([128, 512], F32, tag="W", bufs=2, name="pvg")
            for dc in range(8):
                nc.tensor.matmul(pvg[0:32, :], xg[:, dc, :], wv[:, dc, :],
                                 start=(dc == 0), stop=(dc == 7))
            vgP = sb.tile([128, 4, 65], BF16, tag="vgP", name="vgP")
            for h in range(8):
                p, hh = h // 2, h % 2
                nc.scalar.copy(vgP[32 * hh:32 * hh + 32, p, 0:64],
                               pvg[0:32, 64 * h:64 * h + 64])
            nc.gpsimd.memset(vgP[:, :, 64], 1.0)

            QTg, KTg = [], []
            for p in range(4):
                pgE = ps.tile([128, 512], F32, tag="W", bufs=2, name=f"pgE{p}")
                for dc in range(8):
                    nc.tensor.matmul(pgE[:, 0:32], wqk[:, dc, p, 0, :],
                                     xg[:, dc, :], start=(dc == 0),
                                     stop=(dc == 7))
                pgO = ps.tile([128, 512], F32, tag="W", bufs=2, name=f"pgO{p}")
                for dc in range(8):
                    nc.tensor.matmul(pgO[:, 0:32], wqk[:, dc, p, 1, :],
                                     xg[:, dc, :], start=(dc == 0),
                                     stop=(dc == 7))
                Eg = sb.tile([128, 32], BF16, tag="eog", bufs=4, name=f"Eg{p}")
                nc.scalar.copy(Eg[:], pgE[:, 0:32])
                Og = sb.tile([128, 32], BF16, tag="eog", bufs=4, name=f"Og{p}")
                nc.scalar.copy(Og[:], pgO[:, 0:32])
                t1 = sb.tile([128, 32], BF16, tag="rtg", bufs=4, name=f"g1{p}")
                t2 = sb.tile([128, 32], BF16, tag="rtg", bufs=4, name=f"g2{p}")
                nc.vector.tensor_mul(t1[:], Eg[:], cosg[:])
                nc.vector.tensor_mul(t2[:], Og[:], sing[:])
                REg = sb.tile([128, 32], BF16, tag="reg", bufs=2, name=f"REg{p}")
                nc.vector.tensor_sub(REg[:], t1[:], t2[:])
                t3 = sb.tile([128, 32], BF16, tag="rtg", bufs=4, name=f"g3{p}")
                t4 = sb.tile([128, 32], BF16, tag="rtg", bufs=4, name=f"g4{p}")
                nc.vector.tensor_mul(t3[:], Eg[:], sing[:])
                nc.vector.tensor_mul(t4[:], Og[:], cosg[:])
                ROg = sb.tile([128, 32], BF16, tag="rog", bufs=2, name=f"ROg{p}")
                nc.vector.tensor_add(ROg[:], t3[:], t4[:])
                qtg = sb.tile([128, 32], BF16, tag="QTg", bufs=4, name=f"QTg{p}")
                ktg = sb.tile([128, 32], BF16, tag="KTg", bufs=4, name=f"KTg{p}")
                extract(qtg, ktg, REg, ROg, 32)
                QTg.append(qtg)
                KTg.append(ktg)

            p2sb = [sb.tile([32, 65], F32, tag="p2sb", bufs=8, name=f"p2sb{h}")
                    for h in range(8)]
            KTkeep = [sb.tile([128, 128], BF16, tag="ktk", bufs=4,
                              name=f"ktk{p}") for p in range(4)]
            eb_prev = [None] * 8
            vaug_prev7 = None

            # ---------- main loop over halves ----------
            for Hh in range(2):
                t0 = HALF * Hh

                # V projection for this half
                vaug = []
                for jh in range(4):
                    pv = ps.tile([128, 1024], F32, tag="X", bufs=2,
                                 name=f"pv{Hh}_{jh}")
                    for s2 in range(2):
                        jl = 2 * jh + s2
                        r0 = t0 + 128 * jl
                        for dc in range(8):
                            nc.tensor.matmul(
                                pv[:, 512 * s2:512 * s2 + 512],
                                xt[:, dc, r0:r0 + 128], wv[:, dc, :],
                                start=(dc == 0), stop=(dc == 7))
                    for s2 in range(2):
                        jl = 2 * jh + s2
                        va = sb.tile([128, 8, 65], BF16, tag="vaug", bufs=9,
                                     name=f"va{Hh}_{jl}")
                        nc.scalar.copy(
                            va[:, :, 0:64],
                            pv[:, 512 * s2:512 * s2 + 512]
                            .rearrange("p (a b) -> p a b", a=8))
                        nc.gpsimd.memset(va[:, :, 64], 1.0)
                        vaug.append(va)

                attn = [sb.tile([128, 1024], BF16, tag="attn", bufs=4,
                                name=f"attn{Hh}_{cc}") for cc in range(4)]

                # pipelined pair loop: projection of pair p emitted before
                # scores of pair p-1 so TensorE stays fed during rope
                projected = {}

                def proj_pair(p):
                    E = sb.tile([128, 1024], BF16, tag="eo", bufs=4,
                                name=f"E{Hh}_{p}")
                    O = sb.tile([128, 1024], BF16, tag="eo", bufs=4,
                                name=f"O{Hh}_{p}")
                    for tcc in range(2):
                        P1 = ps.tile([128, 512], F32, tag="W", bufs=2,
                                     name=f"P1_{Hh}_{p}_{tcc}")
                        for dc in range(8):
                            nc.tensor.matmul(
                                P1[:], wqk[:, dc, p, 0, :],
                                xt[:, dc, t0 + 512 * tcc:t0 + 512 * tcc + 512],
                                start=(dc == 0), stop=(dc == 7))
                        nc.scalar.copy(E[:, 512 * tcc:512 * tcc + 512], P1[:])
                        P2 = ps.tile([128, 512], F32, tag="W", bufs=2,
                                     name=f"P2_{Hh}_{p}_{tcc}")
                        for dc in range(8):
                            nc.tensor.matmul(
                                P2[:], wqk[:, dc, p, 1, :],
                                xt[:, dc, t0 + 512 * tcc:t0 + 512 * tcc + 512],
                                start=(dc == 0), stop=(dc == 7))
                        nc.scalar.copy(O[:, 512 * tcc:512 * tcc + 512], P2[:])
                    # rope
                    cs = cosT[:, t0:t0 + 1024]
                    sn = sinT[:, t0:t0 + 1024]
                    t1 = sb.tile([128, 1024], BF16, tag="rt", bufs=4,
                                 name=f"t1{Hh}_{p}")
                    t2 = sb.tile([128, 1024], BF16, tag="rt", bufs=4,
                                 name=f"t2{Hh}_{p}")
                    nc.vector.tensor_mul(t1[:], E[:], cs)
                    nc.vector.tensor_mul(t2[:], O[:], sn)
                    RE = sb.tile([128, 1024], BF16, tag="re", bufs=2,
                                 name=f"RE{Hh}_{p}")
                    nc.vector.tensor_sub(RE[:], t1[:], t2[:])
                    t3 = sb.tile([128, 1024], BF16, tag="rt", bufs=4,
                                 name=f"t3{Hh}_{p}")
                    t4 = sb.tile([128, 1024], BF16, tag="rt", bufs=4,
                                 name=f"t4{Hh}_{p}")
                    nc.vector.tensor_mul(t3[:], E[:], sn)
                    nc.vector.tensor_mul(t4[:], O[:], cs)
                    RO = sb.tile([128, 1024], BF16, tag="ro", bufs=2,
                                 name=f"RO{Hh}_{p}")
                    nc.vector.tensor_add(RO[:], t3[:], t4[:])
                    QT = sb.tile([128, 1024], BF16, tag="QT", bufs=3,
                                 name=f"QT{Hh}_{p}")
                    KT = sb.tile([128, 1024], BF16, tag="KT", bufs=3,
                                 name=f"KT{Hh}_{p}")
                    extract(QT, KT, RE, RO, 1024)
                    projected[p] = (QT, KT)

                proj_pair(0)
                for p in range(4):
                    if p + 1 < 4:
                        proj_pair(p + 1)
                    QT, KT = projected.pop(p)

                    # global-key scores for both heads of the pair
                    EG = sb.tile([64, 1024], BF16, tag="EG", bufs=2,
                                 name=f"EG{Hh}_{p}")
                    for tcc in range(2):
                        psG = ps.tile([64, 512], F32, tag="G", bufs=1,
                                      name=f"psG{Hh}_{p}_{tcc}")
                        sl = slice(512 * tcc, 512 * tcc + 512)
                        nc.tensor.matmul(psG[0:32, :], KTg[p][0:64, :],
                                         QT[0:64, sl], start=True, stop=True)
                        nc.tensor.matmul(psG[32:64, :], KTg[p][64:128, :],
                                         QT[64:128, sl], start=True, stop=True)
                        nc.scalar.activation(EG[:, sl], psG[:], EXP,
                                             scale=EXP_SCALE)
                        nc.gpsimd.tensor_mul(EG[:, sl], EG[:, sl],
                                             mglob[:, t0 + 512 * tcc:
                                                   t0 + 512 * tcc + 512])

                    ejs_h = {}
                    for hh in range(2):
                        h = 2 * p + hh
                        bs = 64 * hh

                        # window scores (batched exp per 2 j-chunk bands)
                        ejs = []
                        for jpp in range(2):
                            pw = ps.tile([128, 1024], F32, tag="X", bufs=2,
                                         name=f"pw{Hh}_{h}_{jpp}")
                            for sub in range(2):
                                j0 = 2 * (2 * jpp + sub)
                                j1 = j0 + 1
                                w1 = 256 if j1 < 7 else 128
                                c0 = 512 * sub
                                nc.tensor.matmul(
                                    pw[:, c0:c0 + 256],
                                    KT[bs:bs + 64, 128 * j0:128 * j0 + 128],
                                    QT[bs:bs + 64, 128 * j0:128 * j0 + 256],
                                    start=True, stop=True)
                                nc.tensor.matmul(
                                    pw[:, c0 + 256:c0 + 256 + w1],
                                    KT[bs:bs + 64, 128 * j1:128 * j1 + 128],
                                    QT[bs:bs + 64, 128 * j1:128 * j1 + w1],
                                    start=True, stop=True)
                            wdt = 1024 if jpp == 0 else 896
                            ej = sb.tile([128, 1024], BF16, tag="ej", bufs=5,
                                         name=f"ej{Hh}_{h}_{jpp}")
                            nc.scalar.activation(ej[:, 0:wdt], pw[:, 0:wdt],
                                                 EXP, scale=EXP_SCALE)
                            nc.vector.tensor_mul(ej[:, 0:wdt], ej[:, 0:wdt],
                                                 mwin[:, 0:wdt])
                            ejs.append(ej)

                        if Hh == 1:
                            pb = ps.tile([128, 512], F32, tag="W", bufs=2,
                                         name=f"pb{h}")
                            nc.tensor.matmul(pb[:, 0:128],
                                             KTkeep[p][bs:bs + 64, :],
                                             QT[bs:bs + 64, 0:128],
                                             start=True, stop=True)
                            eb = sb.tile([128, 128], BF16, tag="eb", bufs=2,
                                         name=f"eb{h}")
                            nc.scalar.activation(eb[:], pb[:, 0:128], EXP,
                                                 scale=EXP_SCALE)
                            nc.vector.tensor_mul(eb[:], eb[:],
                                                 mwin[:, 128:256])
                            eb_prev[h] = eb
                        ejs_h[hh] = ejs

                    for hh in range(2):
                        h = 2 * p + hh
                        bs = 64 * hh
                        ejs = ejs_h[hh]
                        # AV for this head: both 512-col chunks in one psum
                        U2 = ps.tile([128, 1024], F32, tag="X", bufs=2,
                                     name=f"U{Hh}_{h}")
                        for c in range(2):
                            ub = 512 * c
                            if c == 0:
                                parts = [(0, 0, 256, 0), (1, 128, 384, 0),
                                         (2, 256, 512, 0), (3, 384, 512, 0)]
                            else:
                                parts = [(4, 0, 256, 0), (3, 0, 128, 128),
                                         (5, 128, 384, 0), (6, 256, 512, 0),
                                         (7, 384, 512, 0)]
                            # full-width global-V term first (start=True) so
                            # later overlapping window parts accumulate into
                            # fully-written territory
                            nc.tensor.matmul(U2[0:65, ub:ub + 512],
                                             vgP[32 * hh:32 * hh + 32, p, :],
                                             EG[32 * hh:32 * hh + 32,
                                                ub:ub + 512],
                                             start=True, stop=False)
                            if Hh == 1 and c == 0:
                                nc.tensor.matmul(U2[0:65, 0:128],
                                                 vaug_prev7[:, h, :],
                                                 eb_prev[h][:],
                                                 start=False, stop=False)
                            for n_, (jl, u0, u1, e0c) in enumerate(parts):
                                w = u1 - u0
                                ecol = 256 * (jl % 4) + e0c
                                nc.tensor.matmul(
                                    U2[0:65, ub + u0:ub + u1],
                                    vaug[jl][:, h, :],
                                    ejs[jl // 4][:, ecol:ecol + w],
                                    start=False, stop=(n_ == len(parts) - 1))
                        # softmax normalization: Z row -> SBUF (approx_fast
                        # reads PSUM incorrectly on HW) -> 1/Z -> broadcast
                        Zsb = sb.tile([1, 1024], F32, tag="Zsb", bufs=2,
                                      name=f"Zsb{Hh}_{h}")
                        nc.scalar.copy(Zsb[:], U2[64:65, 0:1024])
                        zrf = sb.tile([1, 1024], F32, tag="zrf", bufs=2,
                                      name=f"zrf{Hh}_{h}")
                        nc.vector.reciprocal_approx_fast(out=zrf[:],
                                                         in_=Zsb[:])
                        for c in range(2):
                            ub = 512 * c
                            bc = sb.tile([64, 512], F32, tag="bc", bufs=3,
                                         name=f"bc{Hh}_{h}_{c}")
                            nc.gpsimd.partition_broadcast(
                                bc[:], zrf[0:1, ub:ub + 512])
                            nc.vector.tensor_mul(
                                attn[p][bs:bs + 64, ub:ub + 512],
                                U2[0:64, ub:ub + 512], bc[:])

                    # p2: global-query scores for both heads (transposed)
                    EPT = sb.tile([64, 1024], BF16, tag="EPT", bufs=2,
                                  name=f"EPT{Hh}_{p}")
                    for tcc in range(2):
                        psP = ps.tile([64, 512], F32, tag="G", bufs=1,
                                      name=f"psP{Hh}_{p}_{tcc}")
                        sl = slice(512 * tcc, 512 * tcc + 512)
                        nc.tensor.matmul(psP[0:32, :], QTg[p][0:64, :],
                                         KT[0:64, sl], start=True, stop=True)
                        nc.tensor.matmul(psP[32:64, :], QTg[p][64:128, :],
                                         KT[64:128, sl], start=True, stop=True)
                        nc.scalar.activation(EPT[:, sl], psP[:], EXP,
                                             scale=EXP_SCALE)
                        nc.vector.tensor_mul(EPT[:, sl], EPT[:, sl],
                                             mp2t[:, t0 + 512 * tcc:
                                                  t0 + 512 * tcc + 512])

                    for hh in range(2):
                        h = 2 * p + hh
                        ptp = ps.tile([128, 256], BF16, tag="T", bufs=1,
                                      name=f"ptp{Hh}_{h}")
                        for jl in range(8):
                            nc.tensor.transpose(
                                ptp[:, 32 * jl:32 * jl + 32],
                                EPT[32 * hh:32 * hh + 32,
                                    128 * jl:128 * jl + 128],
                                idb[32 * hh:32 * hh + 32, :])
                        ep2 = sb.tile([128, 256], BF16, tag="ep2", bufs=2,
                                      name=f"ep2{Hh}_{h}")
                        nc.scalar.copy(ep2[:], ptp[:])
                        pc = ps.tile([64, 512], F32, tag="G", bufs=1,
                                     name=f"pc{Hh}_{h}")
                        for jl in range(8):
                            nc.tensor.matmul(pc[0:32, 0:65],
                                             ep2[:, 32 * jl:32 * jl + 32],
                                             vaug[jl][:, h, :],
                                             start=(jl == 0), stop=(jl == 7))
                        if Hh == 0:
                            nc.vector.tensor_copy(p2sb[h][:], pc[0:32, 0:65])
                        else:
                            nc.vector.tensor_add(p2sb[h][:], p2sb[h][:],
                                                 pc[0:32, 0:65])

                    if Hh == 0:
                        nc.vector.tensor_copy(KTkeep[p][:].bitcast(I32),
                                              KT[:, 896:1024].bitcast(I32))

                if Hh == 0:
                    vaug_prev7 = vaug[7]

                # p2 finalize for this half's global queries
                m0 = 16 * Hh
                for h in range(8):
                    p, hh = h // 2, h % 2
                    rz2 = sb.tile([32, 1], F32, tag="rz2", bufs=2,
                                  name=f"rz2{Hh}_{h}")
                    nc.vector.reciprocal(rz2[:], p2sb[h][:, 64:65])
                    p2n = sb.tile([32, 64], F32, tag="p2n", bufs=2,
                                  name=f"p2n{Hh}_{h}")
                    nc.vector.tensor_scalar_mul(p2n[:], p2sb[h][:, 0:64],
                                                rz2[:])
                    ptf = ps.tile([64, 512], F32, tag="G", bufs=1,
                                  name=f"ptf{Hh}_{h}")
                    nc.tensor.transpose(ptf[0:64, 0:32], p2n[:],
                                        ident[0:32, 0:32])
                    nc.vector.tensor_copy(
                        attn[p][64 * hh:64 * hh + 64, 0::64],
                        ptf[0:64, m0:m0 + 16])

                # out projection for this half
                for tcc in range(8):
                    po = ps.tile([128, 1024], F32, tag="X", bufs=2,
                                 name=f"po{Hh}_{tcc}")
                    for nck in range(2):
                        for cci in range(4):
                            nc.tensor.matmul(
                                po[:, 512 * nck:512 * nck + 512],
                                attn[cci][:, 128 * tcc:128 * tcc + 128],
                                wout[:, cci, 512 * nck:512 * nck + 512],
                                start=(cci == 0), stop=(cci == 3))
                    os_ = sb.tile([128, 1024], BF16, tag="os", bufs=3,
                                  name=f"os{Hh}_{tcc}")
                    nc.scalar.copy(os_[:], po[:])
                    nc.sync.dma_start(
                        out=out_d[t0 + 128 * tcc:t0 + 128 * tcc + 128, :],
                        in_=os_[:])

    nc.compile()
    return nc


def _prep_inputs(x, W_qkv, W_out):
    bf = ml_dtypes.bfloat16

    pos = np.arange(T, dtype=np.float32)
    inv_freq = 1.0 / (10000.0 ** (np.arange(32, dtype=np.float32) / 32.0))
    ang = inv_freq[:, None] * pos[None, :]  # (32, T)
    c32 = np.cos(ang).astype(np.float32)
    s32 = np.sin(ang).astype(np.float32)
    cos128 = np.ascontiguousarray(np.tile(c32, (4, 1))).astype(bf)
    sin128 = np.ascontiguousarray(np.tile(s32, (4, 1))).astype(bf)
    cosgk = np.ascontiguousarray(cos128[:, 0::64])
    singk = np.ascontiguousarray(sin128[:, 0::64])

    s = np.arange(128)[:, None]
    u = np.arange(256)[None, :]
    mwin1 = ((u >= s) & ((u <= s + 127) | (s % 64 == 0))).astype(bf)
    mwin = np.ascontiguousarray(np.tile(mwin1, (1, 4)))
    m = np.arange(32)[:, None]
    q = np.arange(T)[None, :]
    mglob32 = (q >= 128 * (m // 2 + 2)).astype(bf)
    mglob = np.ascontiguousarray(np.tile(mglob32, (2, 1)))
    k = np.arange(T)[None, :]
    mp2t32 = (k <= 64 * m).astype(bf)
    mp2t = np.ascontiguousarray(np.tile(mp2t32, (2, 1)))
    idb = np.ascontiguousarray(np.tile(np.eye(32, dtype=np.float32),
                                       (4, 1))).astype(bf)

    Wq = W_qkv[:, 0:D]
    Wk = W_qkv[:, D:2 * D]

    in_maps = []
    for core in range(NCORES):
        b, g = core // 2, core % 2
        xT = np.ascontiguousarray(x[b].T).astype(bf)  # (1024, 2048)
        xt = np.ascontiguousarray(
            xT.reshape(8, 128, T).transpose(1, 0, 2))  # [dp, dc, t]
        xgt = np.ascontiguousarray(xt[:, :, 0::64])

        cols = np.zeros((D, 4, 2, 128), dtype=np.float32)
        for p in range(4):
            h0 = 8 * g + 2 * p
            h1 = h0 + 1
            for eo in range(2):
                sel = slice(eo, None, 2)
                blk = np.concatenate(
                    [Wq[:, 64 * h0:64 * h0 + 64][:, sel],
                     Wk[:, 64 * h0:64 * h0 + 64][:, sel],
                     Wq[:, 64 * h1:64 * h1 + 64][:, sel],
                     Wk[:, 64 * h1:64 * h1 + 64][:, sel]], axis=1)
                cols[:, p, eo, :] = blk
        wqk = np.ascontiguousarray(
            cols.reshape(8, 128, 4, 2, 128).transpose(1, 0, 2, 3, 4)
        ).astype(bf)

        wvp = np.ascontiguousarray(
            W_qkv[:, 2 * D + 512 * g:2 * D + 512 * (g + 1)]
            .reshape(8, 128, 512).transpose(1, 0, 2)).astype(bf)
        woutp = np.ascontiguousarray(
            W_out[512 * g:512 * (g + 1)]
            .reshape(4, 128, 1024).transpose(1, 0, 2)).astype(bf)

        in_maps.append({
            "xt": xt, "xg": xgt, "wqk": wqk, "wv": wvp, "wout": woutp,
            "cosd": cos128, "sind": sin128, "cosg": cosgk, "sing": singk,
            "mwin": mwin, "mglob": mglob, "mp2t": mp2t, "idb": idb,
        })
    return in_maps


def kernel(x, W_qkv, W_out, b_out):
    x = np.asarray(x, dtype=np.float32)
    W_qkv = np.asarray(W_qkv, dtype=np.float32)
    W_out = np.asarray(W_out, dtype=np.float32)
    b_out = np.asarray(b_out, dtype=np.float32)

    if "nc" not in _cache:
        _cache["nc"] = _build()
    nc = _cache["nc"]

    in_maps = _prep_inputs(x, W_qkv, W_out)
    res = run_bass_kernel_spmd(nc, in_maps, core_ids=list(range(NCORES)))

    out = np.zeros((B, T, D), dtype=np.float32)
    for core in range(NCORES):
        out[core // 2] += np.asarray(res.results[core]["out"],
                                     dtype=np.float32)
    out += b_out[None, None, :]
    return out
